# revision 1
# baseline (speedup 1.0000x reference)
"""Trainium2 Bass kernel for the combined Tacotron-style loss.

Strategy (pure data parallel, 8 samples per core on 8 NeuronCores):

Device (per core, one SPMD program):
  - mel L1 terms:   DVE subtract + ACT Abs with fused row-sum accumulation.
  - gate BCE:       ACT Abs/Exp/Ln + fused DVE multiply-reduce.
  - attention term: rows with i >= T_in have a full-row mask -> handled by a
    PE "selector matmul" (per-tile [128,12] selector weights accumulate
    per-sample/valid-row column sums of `alignments` into PSUM). Rows with
    i < T_in use a fused scalar_tensor_tensor (iota < i) * A with accum.
  - guided attention: sum(A * guided) = box_sum - gauss_sum where
      box_sum  = per-sample column sums (same PE selector matmul, masked by
                 j < in_len on the host afterwards)
      gauss_sum: exp(-(i - j*out/in)^2 / (2*sigma^2)) is a band of <= 4
                 columns per row (sigma = 0.4) -> computed on a host-gathered
                 [rows, 8] band: DVE square, ACT exp, DVE mult, DVE reduce.
Host: shards inputs, builds tiny O(B*T_out) aux tensors, gathers the 8-wide
band, and combines all per-partition partial sums in float64.
"""

import ml_dtypes
import numpy as np

import concourse.bacc as bacc
import concourse.mybir as mybir
from concourse import bass
from concourse.bass_utils import run_bass_kernel_spmd
from concourse.tile import TileContext

F32 = mybir.dt.float32
BF16 = mybir.dt.bfloat16
I32 = mybir.dt.int32
ALU = mybir.AluOpType
ACTF = mybir.ActivationFunctionType

# Problem shapes (hardcoded per contract).
B, MEL, TOUT, TIN = 64, 80, 2000, 400
NCORES = 8
BPC = B // NCORES                    # samples per core
ROWS = BPC * TOUT                    # 16000 (b, i) rows per core
NT = ROWS // 128                     # 125 row tiles
MROWS = BPC * MEL                    # 640 mel rows per core
NMT = MROWS // 128                   # 5 mel tiles
GCOLS = ROWS // 128                  # 125 gate cols ([128, 125] layout)
BW = 8                               # gaussian band width
SIGMA = 0.4
ESCALE = -1.0 / (2.0 * SIGMA * SIGMA)   # -3.125
MEL_W, GATE_W, ATT_W, GA_W = 1.0, 1.0, 0.1, 0.1
SEL_COLS = 12                        # 8 sample cols + 1 att-full col + pad
ALCHUNK = 5                          # align row-tiles per DMA (512 KB bf16)

# Row tiles that contain any row with i = row % TOUT < TIN (att partial pass).
ATT_TILES = sorted(
    t for t in range(NT)
    if any((128 * t + p) % TOUT < TIN for p in range(128))
)
NATT = len(ATT_TILES)

# stats_act cols: 0..4 mel1, 5..9 mel2, 10 gate softplus, 11 gate relu
# stats_dve cols: 0 gate x*z, 1..NATT att partials
SA_COLS = 16
SD_COLS = 48
assert 1 + NATT <= SD_COLS


def _build_program():
    return _build_program_reps(1)


def _build_program_reps(n_reps):
    nc = bacc.Bacc(
        "TRN2",
        target_bir_lowering=False,
        debug=False,
        enable_asserts=False,
        num_devices=NCORES,
    )

    d_melo = nc.dram_tensor("melo", (MROWS, TOUT), BF16, kind="ExternalInput").ap()
    d_melp = nc.dram_tensor("melp", (MROWS, TOUT), BF16, kind="ExternalInput").ap()
    d_melt = nc.dram_tensor("melt", (MROWS, TOUT), BF16, kind="ExternalInput").ap()
    d_go = nc.dram_tensor("go", (128, GCOLS), F32, kind="ExternalInput").ap()
    d_gt = nc.dram_tensor("gt", (128, GCOLS), F32, kind="ExternalInput").ap()
    d_al = nc.dram_tensor("al", (ROWS, TIN), BF16, kind="ExternalInput").ap()
    # sel is pre-transposed on the host to the SBUF layout [128, NT*SEL_COLS]
    d_sel = nc.dram_tensor("sel", (128, NT * SEL_COLS), BF16,
                           kind="ExternalInput").ap()
    d_iatt = nc.dram_tensor("iatt", (128, NT), F32, kind="ExternalInput").ap()
    # band / bandd are pre-transposed on the host to the SBUF layout
    d_band = nc.dram_tensor("band", (128, NT * BW), F32, kind="ExternalInput").ap()
    d_bandd = nc.dram_tensor("bandd", (128, NT * BW), F32, kind="ExternalInput").ap()

    o_sa = nc.dram_tensor("stats_act", (128, SA_COLS), F32, kind="ExternalOutput").ap()
    o_sd = nc.dram_tensor("stats_dve", (128, SD_COLS), F32, kind="ExternalOutput").ap()
    o_br = nc.dram_tensor("band_red", (128, NT), F32, kind="ExternalOutput").ap()
    o_cs = nc.dram_tensor("colsums", (SEL_COLS, TIN), F32, kind="ExternalOutput").ap()

    al_r = d_al.rearrange("(c n p) j -> c p n j", p=128, n=ALCHUNK)
    sel_r = d_sel
    melo_r = d_melo.rearrange("(k p) j -> k p j", p=128)
    melp_r = d_melp.rearrange("(k p) j -> k p j", p=128)
    melt_r = d_melt.rearrange("(k p) j -> k p j", p=128)

    with TileContext(nc) as tc:
        with (
            tc.tile_pool(name="alp", bufs=6) as alp,
            tc.tile_pool(name="melpool", bufs=2) as mpool,
            tc.tile_pool(name="scr", bufs=2) as scr,
            tc.tile_pool(name="small", bufs=1) as sp,
            tc.tile_pool(name="psum", bufs=1, space="PSUM") as psp,
        ):
            # --- constants / aux ---
            iota400_i = sp.tile([128, TIN], I32)
            nc.gpsimd.iota(iota400_i[:], pattern=[[1, TIN]], base=0,
                           channel_multiplier=0)
            iota400 = sp.tile([128, TIN], F32)
            nc.vector.tensor_copy(out=iota400[:], in_=iota400_i[:])

            for _rep in range(n_reps):
                _emit_body(nc, alp, mpool, scr, sp, psp, iota400,
                           sel_r, d_iatt, d_go, d_gt, d_band, d_bandd,
                           al_r, melo_r, melp_r, melt_r,
                           o_sa, o_sd, o_br, o_cs)

    nc.compile()
    return nc


def _emit_body(nc, alp, mpool, scr, sp, psp, iota400,
               sel_r, d_iatt, d_go, d_gt, d_band, d_bandd,
               al_r, melo_r, melp_r, melt_r,
               o_sa, o_sd, o_br, o_cs):
    if True:
        if True:
            sel_flat = sp.tile([128, NT * SEL_COLS], BF16)
            nc.sync.dma_start(out=sel_flat[:], in_=sel_r)
            sel_sb = sel_flat[:].rearrange("p (t k) -> p t k", k=SEL_COLS)
            iatt_sb = sp.tile([128, NT], F32)
            nc.sync.dma_start(out=iatt_sb[:], in_=d_iatt)

            stats_act = sp.tile([128, SA_COLS], F32)
            nc.vector.memset(stats_act[:], 0.0)
            stats_dve = sp.tile([128, SD_COLS], F32)
            nc.vector.memset(stats_dve[:], 0.0)

            cs_ps = psp.tile([SEL_COLS, TIN], F32)

            att_col = {t: 1 + k for k, t in enumerate(ATT_TILES)}

            # --- gate BCE (small; emit early so ACT/DVE have warmup work) ---
            go = sp.tile([128, GCOLS], F32)
            nc.sync.dma_start(out=go[:], in_=d_go)
            gt = sp.tile([128, GCOLS], F32)
            nc.sync.dma_start(out=gt[:], in_=d_gt)
            gs1 = sp.tile([128, GCOLS], F32)
            gs2 = sp.tile([128, GCOLS], F32)
            # softplus(-|x|) = ln(1 + exp(-|x|))
            nc.scalar.activation(out=gs1[:], in_=go[:], func=ACTF.Abs)
            nc.scalar.activation(out=gs2[:], in_=gs1[:], func=ACTF.Exp, scale=-1.0)
            nc.scalar.activation(out=gs1[:], in_=gs2[:], func=ACTF.Ln, bias=1.0,
                                 accum_out=stats_act[:, 10:11])
            nc.scalar.activation(out=gs2[:], in_=go[:], func=ACTF.Relu,
                                 accum_out=stats_act[:, 11:12])
            gs3 = sp.tile([128, GCOLS], F32)
            nc.vector.scalar_tensor_tensor(
                out=gs3[:], in0=go[:], scalar=0.0, in1=gt[:],
                op0=ALU.add, op1=ALU.mult, accum_out=stats_dve[:, 0:1],
            )

            # --- gaussian band ---
            band_sb = sp.tile([128, NT * BW], F32)
            nc.sync.dma_start(out=band_sb[:], in_=d_band)
            bandd_sb = sp.tile([128, NT * BW], F32)
            nc.sync.dma_start(out=bandd_sb[:], in_=d_bandd)
            w1 = sp.tile([128, NT * BW], F32)
            nc.vector.tensor_mul(out=w1[:], in0=bandd_sb[:], in1=bandd_sb[:])
            nc.scalar.activation(out=w1[:], in_=w1[:], func=ACTF.Exp, scale=ESCALE)
            nc.vector.tensor_mul(out=w1[:], in0=w1[:], in1=band_sb[:])
            br = sp.tile([128, NT], F32)
            nc.vector.tensor_reduce(
                out=br[:],
                in_=w1[:].rearrange("p (t f) -> p t f", f=BW),
                axis=mybir.AxisListType.X,
                op=ALU.add,
            )
            nc.sync.dma_start(out=o_br, in_=br[:])

            # --- align pass (PE selector matmuls + att partial rows),
            #     with the mel pass interleaved so every engine stays fed ---
            mel_stream = []
            for k in range(NMT):
                mel_stream.append((melt_r[k], melo_r[k], melp_r[k], k))

            def emit_mel(step):
                mt_r, mo_r, mp_r, k = mel_stream[step]
                mt = mpool.tile([128, TOUT], BF16, tag="mt")
                nc.sync.dma_start(out=mt[:], in_=mt_r)
                mo = mpool.tile([128, TOUT], BF16, tag="mo")
                nc.sync.dma_start(out=mo[:], in_=mo_r)
                mp = mpool.tile([128, TOUT], BF16, tag="mp")
                nc.sync.dma_start(out=mp[:], in_=mp_r)
                s1 = scr.tile([128, TOUT], F32, tag="melscr")
                nc.vector.tensor_sub(out=s1[:], in0=mo[:], in1=mt[:])
                nc.scalar.activation(out=s1[:], in_=s1[:], func=ACTF.Abs,
                                     accum_out=stats_act[:, k:k + 1])
                s2 = scr.tile([128, TOUT], F32, tag="melscr2")
                nc.vector.tensor_sub(out=s2[:], in0=mp[:], in1=mt[:])
                nc.scalar.activation(out=s2[:], in_=s2[:], func=ACTF.Abs,
                                     accum_out=stats_act[:, NMT + k:NMT + k + 1])

            nch = NT // ALCHUNK
            mel_every = nch // NMT   # one mel tile-trio per 5 align chunks
            for c in range(nch):
                if c % mel_every == mel_every // 2:
                    emit_mel(c // mel_every)
                a = alp.tile([128, ALCHUNK, TIN], BF16, tag="a")
                nc.sync.dma_start(out=a[:], in_=al_r[c])
                for n in range(ALCHUNK):
                    t = c * ALCHUNK + n
                    nc.tensor.matmul(
                        cs_ps[:],
                        sel_sb[:, t, :],
                        a[:, n, :],
                        start=(t == 0),
                        stop=(t == NT - 1),
                    )
                    if t in att_col:
                        sc = scr.tile([128, TIN], BF16, tag="attscr")
                        k = att_col[t]
                        nc.vector.scalar_tensor_tensor(
                            out=sc[:],
                            in0=iota400[:],
                            scalar=iatt_sb[:, t:t + 1],
                            in1=a[:, n, :],
                            op0=ALU.is_lt,
                            op1=ALU.mult,
                            accum_out=stats_dve[:, k:k + 1],
                        )

            # --- outputs ---
            cs_sb = sp.tile([SEL_COLS, TIN], F32)
            nc.scalar.copy(out=cs_sb[:], in_=cs_ps[:])
            nc.sync.dma_start(out=o_cs, in_=cs_sb[:])
            nc.sync.dma_start(out=o_sa, in_=stats_act[:])
            nc.sync.dma_start(out=o_sd, in_=stats_dve[:])


_PROGRAM = None


def _get_program():
    global _PROGRAM
    if _PROGRAM is None:
        _PROGRAM = _build_program()
    return _PROGRAM


def _to_pt(a):
    """[ROWS] or [ROWS, W] row-major -> [128, ...] SBUF layout where
    partition p, tile t holds row 128 t + p."""
    if a.ndim == 1:
        return np.ascontiguousarray(a.reshape(NT, 128).T, dtype=np.float32)
    w = a.shape[1]
    return np.ascontiguousarray(
        a.reshape(NT, 128, w).transpose(1, 0, 2).reshape(128, NT * w),
        dtype=np.float32)


def _prep_core(al, melo, melp, melt, go, gt, in_len, out_len):
    """Build one core's input map. al: [BPC, TOUT, TIN] etc. (numpy f32)."""
    al2 = np.ascontiguousarray(al.reshape(ROWS, TIN), dtype=np.float32)

    rows = np.arange(ROWS)
    bi = rows // TOUT                       # sample index within core
    ii = rows % TOUT                        # decoder step i
    inl = in_len[bi].astype(np.float64)     # per-row input length
    outl = out_len[bi].astype(np.float64)   # per-row output length
    rowvalid = ii < outl

    # selector weights for the PE matmul (0/1 -> exact in bf16),
    # pre-transposed to the [128, NT*SEL_COLS] SBUF layout
    sel = np.zeros((ROWS, SEL_COLS), dtype=np.float32)
    for s in range(BPC):
        sel[:, s] = (bi == s) & rowvalid
    sel[:, 8] = ii >= TIN
    sel = np.ascontiguousarray(
        sel.reshape(NT, 128, SEL_COLS).transpose(1, 0, 2).reshape(128, -1)
        .astype(ml_dtypes.bfloat16))

    # att partial-row mask threshold (0 disables the row)
    iatt = np.where(ii < TIN, ii, 0).astype(np.float32)

    # gaussian band: j in [s0, s0+BW) covers |i - j*out/in| <= ~4
    jstar = ii * inl / outl
    s0 = np.clip(np.floor(jstar).astype(np.int64) - 3, 0, TIN - BW)
    jband = s0[:, None] + np.arange(BW)[None, :]          # [ROWS, BW]
    band = al2[rows[:, None], jband].astype(np.float32)
    # d = i - expected_j in the reference's f32 evaluation order
    expected = ((jband.astype(np.float32) / inl[:, None].astype(np.float32))
                * outl[:, None].astype(np.float32))
    bandd = (ii[:, None].astype(np.float32) - expected).astype(np.float32)
    # poison invalid band positions (j >= in_len) so exp() underflows to 0
    bandd[jband >= in_len[bi][:, None]] = 1.0e9

    bf = ml_dtypes.bfloat16
    return {
        "melo": np.ascontiguousarray(melo.reshape(MROWS, TOUT).astype(bf)),
        "melp": np.ascontiguousarray(melp.reshape(MROWS, TOUT).astype(bf)),
        "melt": np.ascontiguousarray(melt.reshape(MROWS, TOUT).astype(bf)),
        "go": np.ascontiguousarray(go.reshape(128, GCOLS), np.float32),
        "gt": np.ascontiguousarray(gt.reshape(128, GCOLS), np.float32),
        "al": al2.astype(bf),
        "sel": sel,
        "iatt": _to_pt(iatt),
        "band": _to_pt(band),
        "bandd": _to_pt(bandd),
    }


def kernel(mel_out, mel_out_postnet, gate_out, alignments,
           mel_target, gate_target, input_lengths, output_lengths,
           _results_hook=None):
    nc = _get_program()

    mel_out = np.asarray(mel_out, dtype=np.float32)
    mel_out_postnet = np.asarray(mel_out_postnet, dtype=np.float32)
    gate_out = np.asarray(gate_out, dtype=np.float32)
    alignments = np.asarray(alignments, dtype=np.float32)
    mel_target = np.asarray(mel_target, dtype=np.float32)
    gate_target = np.asarray(gate_target, dtype=np.float32)
    input_lengths = np.asarray(input_lengths)
    output_lengths = np.asarray(output_lengths)

    in_maps = []
    for c in range(NCORES):
        sl = slice(BPC * c, BPC * (c + 1))
        in_maps.append(_prep_core(
            alignments[sl], mel_out[sl], mel_out_postnet[sl], mel_target[sl],
            gate_out[sl], gate_target[sl],
            input_lengths[sl].astype(np.int64), output_lengths[sl].astype(np.int64),
        ))

    res = run_bass_kernel_spmd(nc, in_maps, core_ids=list(range(NCORES)))
    if _results_hook is not None:
        _results_hook(res)

    mel1 = mel2 = gsp = grelu = gxz = att = box = gauss = 0.0
    for c in range(NCORES):
        out = res.results[c]
        sa = out["stats_act"].astype(np.float64)
        sd = out["stats_dve"].astype(np.float64)
        cs = out["colsums"].astype(np.float64)
        br = out["band_red"].astype(np.float64)

        mel1 += sa[:, 0:NMT].sum()
        mel2 += sa[:, NMT:2 * NMT].sum()
        gsp += sa[:, 10].sum()
        grelu += sa[:, 11].sum()
        gxz += sd[:, 0].sum()
        att += sd[:, 1:1 + NATT].sum() + cs[8, :].sum()

        in_len = input_lengths[BPC * c:BPC * (c + 1)].astype(np.int64)
        out_len = output_lengths[BPC * c:BPC * (c + 1)].astype(np.int64)
        for s in range(BPC):
            box += cs[s, :in_len[s]].sum()

        # band_red[p, t] is the row (128 t + p) gauss partial
        red_flat = br.T.reshape(ROWS)
        rows = np.arange(ROWS)
        bi = rows // TOUT
        ii = rows % TOUT
        valid = ii < out_len[bi]
        gauss += red_flat[valid].sum()

    n_mel = B * MEL * TOUT
    n_gate = B * TOUT
    mel_loss = mel1 / n_mel + mel2 / n_mel
    gate_loss = (grelu - gxz + gsp) / n_gate
    att_loss = att / B
    ga_loss = (box - gauss) / B
    total = (MEL_W * mel_loss + GATE_W * gate_loss
             + ATT_W * att_loss + GA_W * ga_loss)
    f = np.float32
    return (f(total), f(mel_loss), f(gate_loss), f(att_loss), f(ga_loss))



# revision 7
# speedup vs baseline: 2.4842x; 2.4842x over previous
"""Trainium2 Bass kernel for the combined Tacotron-style loss.

Strategy (pure data parallel, 8 samples per core on 8 NeuronCores).

Every loss term is a big reduction, so the kernel is built around moving as
few HBM bytes as possible and reducing them on the widest engines:

  - mel L1 terms: mo/mt/mp stream in fp8 (statistically safe for a 10M-element
    mean at 2e-2 tol).  The PE computes (mo-mt) and (mt-mp) with a +I/-I
    DoubleRow fp8 matmul into PSUM f32; ACT (Abs + accum) and DVE
    (tensor_reduce abs-add) split the row-sum work.
  - attention / guided-attention box terms: alignment rows are normalized
    (sum_j A[i,j] == 1), so sums over wide row prefixes are computed as
    1 - (narrow tail sum).  The host packs exactly the needed tail/window
    elements into a [128, D*512] fp8 "canvas"; a ones-stationary DoubleRow
    matmul chain column-sums it into one PSUM bank.  Column index mod 512
    identifies the group (box-tail / att-direct / att-tail) on the host.
  - gaussian term: sigma=0.4 makes exp(-(i-j*out/in)^2/(2s^2)) a <=4-column
    band; host gathers band values + weights, one DVE mult+accum reduces it.
  - gate BCE: f32, ACT Abs/Exp/Ln + Relu + DVE x*z, all with fused accum.

Host combines all partial sums in float64.
"""

import ml_dtypes
import numpy as np

import concourse.bacc as bacc
import concourse.mybir as mybir
from concourse import bass
from concourse.bass_utils import run_bass_kernel_spmd
from concourse.tile import TileContext

F32 = mybir.dt.float32
BF16 = mybir.dt.bfloat16
F8 = mybir.dt.float8e4
ALU = mybir.AluOpType
ACTF = mybir.ActivationFunctionType
DR = mybir.MatmulPerfMode.DoubleRow

F8NP = ml_dtypes.float8_e4m3
BFNP = ml_dtypes.bfloat16

# Problem shapes (hardcoded per contract).
B, MEL, TOUT, TIN = 64, 80, 2000, 400
NCORES = 8
BPC = B // NCORES                  # samples per core
MROWS = BPC * MEL                  # 640 mel rows per core
NMT = MROWS // 128                 # 5 mel row-tiles
GCOLS = BPC * TOUT // 128          # 125 gate cols ([128, 125] layout)
BW = 4                             # gaussian band width
SIGMA = 0.4
ESCALE = -1.0 / (2.0 * SIGMA * SIGMA)
MEL_W, GATE_W, ATT_W, GA_W = 1.0, 1.0, 0.1, 0.1
ASCALE = 16384.0                   # 2**14: puts fp8 alignment values in normal range

IMID = TIN // 2                    # 200: att rows i<=IMID summed directly,
#                                    i>IMID via 1 - tail
N_DIR = BPC * (IMID * (IMID + 1) // 2)          # direct window elements/core
N_ATT_TAIL = BPC * ((IMID - 1) * IMID // 2)     # att tail elements/core
ATT_CONST = (TOUT - TIN) + (TIN - 1 - IMID)     # exact-1.0 rows per sample

# att-direct mask: rows i=0..IMID, cols j<i  (j <= IMID-1)
_DIR_MASK = np.arange(IMID)[None, :] < np.arange(IMID + 1)[:, None]
# att-tail mask: rows i=IMID+1..TIN-1, cols j>=i
_TAIL_MASK = (np.arange(TIN)[None, :]
              >= (IMID + 1 + np.arange(TIN - 1 - IMID))[:, None])

# mel slot -> engine: even slots ACT, odd slots DVE (10 each)
N_MEL_SLOTS = NMT * 4              # 5 tiles x 2 pairs x 2 chunks
SA_COLS = 16                       # ACT stats: 0..9 mel, 10 softplus, 11 relu
SD_COLS = 16                       # DVE stats: 0..9 mel, 10 x*z, 11 band

# Canvas layout: set lazily from the actual inputs (sizes depend on
# input/output lengths).  (n_chunks D, (a,b) col ranges per group, band cols)
_LAYOUT = None


def _canvas_layout(max_box, nb_cols):
    """Pick D (512-col canvas chunks) + column ranges for the 3 groups."""
    sizes = [max_box, N_DIR, N_ATT_TAIL]
    total = sum(sizes)
    d = max(2, -(-total // (128 * 512)))
    while True:
        cols = [-(-s // (128 * d)) for s in sizes]
        if sum(cols) <= 512:
            break
        d += 1
    d = -(-d // 2) * 2             # even: canvas is consumed in chunk pairs
    ranges = []
    a = 0
    for c in cols:
        ranges.append((a, a + c))
        a += c
    nb = -(-nb_cols // 64) * 64
    return (d, tuple(ranges), nb)


def _build_program(d_chunks, nb, n_reps=1):
    nc = bacc.Bacc(
        "TRN2",
        target_bir_lowering=False,
        debug=False,
        enable_asserts=False,
        num_devices=NCORES,
    )

    d_id = nc.dram_tensor("idw", (128, 256), F8, kind="ExternalInput").ap()
    d_mel = nc.dram_tensor("mel", (128, NMT * 3 * TOUT), F8,
                           kind="ExternalInput").ap()
    d_cv = nc.dram_tensor("cv", (128, d_chunks * 512), F8,
                          kind="ExternalInput").ap()
    d_band = nc.dram_tensor("band", (128, nb), BF16, kind="ExternalInput").ap()
    d_bw = nc.dram_tensor("bw", (128, nb), BF16, kind="ExternalInput").ap()
    d_gate = nc.dram_tensor("gate", (128, 2 * GCOLS), F32,
                            kind="ExternalInput").ap()

    o_sa = nc.dram_tensor("sa", (128, SA_COLS), F32, kind="ExternalOutput").ap()
    o_sd = nc.dram_tensor("sd", (128, SD_COLS), F32, kind="ExternalOutput").ap()
    o_cs = nc.dram_tensor("cs", (1, 512), F32, kind="ExternalOutput").ap()

    with TileContext(nc) as tc:
        with (
            tc.tile_pool(name="small", bufs=1) as sp,
            tc.tile_pool(name="cvp", bufs=3) as cvp,
            tc.tile_pool(name="melp", bufs=3) as melp,
            tc.tile_pool(name="scrp", bufs=2) as scrp,
            tc.tile_pool(name="pscs", bufs=1, space="PSUM") as pscs,
            tc.tile_pool(name="psmel", bufs=3, space="PSUM") as psmel,
        ):
            id_sb = sp.tile([128, 256], F8)
            nc.sync.dma_start(out=id_sb[:], in_=d_id)
            gate_sb = sp.tile([128, 2 * GCOLS], F32)
            nc.sync.dma_start(out=gate_sb[:], in_=d_gate)
            band_sb = sp.tile([128, nb], BF16)
            nc.sync.dma_start(out=band_sb[:], in_=d_band)
            bw_sb = sp.tile([128, nb], BF16)
            nc.sync.dma_start(out=bw_sb[:], in_=d_bw)

            # ones stationary for canvas colsums: DoubleRow requires the
            # k-pair dim stride to be a multiple of 16
            ones2 = sp.tile([128, 32], F8)
            nc.vector.memset(ones2[:], 1.0)
            sa = sp.tile([128, SA_COLS], F32)
            nc.vector.memset(sa[:], 0.0)
            sd = sp.tile([128, SD_COLS], F32)
            nc.vector.memset(sd[:], 0.0)

            cs_ps = pscs.tile([1, 512], F32)

            for _rep in range(n_reps):
                _emit_body(nc, sp, cvp, melp, scrp, psmel,
                           id_sb, gate_sb, band_sb, bw_sb, ones2, sa, sd,
                           cs_ps, d_cv, d_mel, d_chunks, _rep == 0)

            cs_sb = sp.tile([1, 512], F32)
            nc.scalar.copy(out=cs_sb[:], in_=cs_ps[:])
            nc.sync.dma_start(out=o_cs, in_=cs_sb[:])
            nc.sync.dma_start(out=o_sa, in_=sa[:])
            nc.sync.dma_start(out=o_sd, in_=sd[:])

    nc.compile()
    return nc


def _emit_body(nc, sp, cvp, melp, scrp, psmel,
               id_sb, gate_sb, band_sb, bw_sb, ones2, sa, sd,
               cs_ps, d_cv, d_mel, d_chunks, first_rep):
    nb = band_sb.shape[1]

    # --- gate BCE (small; early so ACT/DVE have warmup work) ---
    go = gate_sb[:, 0:GCOLS]
    gt = gate_sb[:, GCOLS:2 * GCOLS]
    g1 = sp.tile([128, GCOLS], F32, tag="g1")
    g2 = sp.tile([128, GCOLS], F32, tag="g2")
    g3 = sp.tile([128, GCOLS], F32, tag="g3")
    g4 = sp.tile([128, GCOLS], F32, tag="g4")
    # softplus(-|x|) = ln(1 + exp(-|x|))
    nc.scalar.activation(out=g1[:], in_=go, func=ACTF.Abs)
    nc.scalar.activation(out=g2[:], in_=g1[:], func=ACTF.Exp, scale=-1.0)
    nc.scalar.activation(out=g3[:], in_=g2[:], func=ACTF.Ln, bias=1.0,
                         accum_out=sa[:, 10:11])
    nc.scalar.activation(out=g4[:], in_=go, func=ACTF.Relu,
                         accum_out=sa[:, 11:12])
    g5 = sp.tile([128, GCOLS], F32, tag="g5")
    nc.vector.scalar_tensor_tensor(
        out=g5[:], in0=go, scalar=0.0, in1=gt,
        op0=ALU.add, op1=ALU.mult, accum_out=sd[:, 10:11])

    # --- gaussian band: sum(band * w) ---
    bscr = sp.tile([128, nb], BF16, tag="bscr")
    nc.vector.scalar_tensor_tensor(
        out=bscr[:], in0=band_sb[:], scalar=1.0, in1=bw_sb[:],
        op0=ALU.mult, op1=ALU.mult, accum_out=sd[:, 11:12])

    ones_v = ones2[:].rearrange("p (two s) -> p two s", two=2)[:, :, 0:1]
    id2 = id_sb[:].rearrange("p (two m) -> p two m", two=2)

    # --- canvas column sums (ones-stationary DoubleRow chain) ---
    n_pairs = d_chunks // 2
    pair = 0
    off = 0
    while off < d_chunks * 512:
        w = min(2048, d_chunks * 512 - off)
        cvt = cvp.tile([128, 2048], F8, tag="cv")
        nc.sync.dma_start(out=cvt[:, 0:w], in_=d_cv[:, off:off + w])
        for h in range(w // 1024):
            rv = cvt[:, h * 1024:(h + 1) * 1024].rearrange(
                "p (two j) -> p two j", two=2)
            nc.tensor.matmul(
                cs_ps[:], ones_v, rv,
                start=(pair == 0),
                stop=(pair == n_pairs - 1),
                perf_mode=DR,
            )
            pair += 1
        off += w

    # --- mel L1: PE computes diffs, ACT/DVE abs+row-sum ---
    half = TOUT // 2
    for k in range(NMT):
        mt = melp.tile([128, 3 * TOUT], F8, tag="mel")
        nc.sync.dma_start(out=mt[:], in_=d_mel[:, k * 3 * TOUT:(k + 1) * 3 * TOUT])
        for p in range(2):
            # pair 0: planes (mo, mt) -> mo - mt; pair 1: (mt, mp) -> mt - mp
            pv = mt[:, p * TOUT:p * TOUT + 2 * TOUT].rearrange(
                "p (two j) -> p two j", two=2)
            for ch in range(2):
                ps = psmel.tile([128, 1024], F32, tag="mps")
                base = ch * half
                nc.tensor.matmul(ps[:, 0:512], id2, pv[:, :, base:base + 512],
                                 start=True, stop=True, perf_mode=DR)
                nc.tensor.matmul(ps[:, 512:half], id2,
                                 pv[:, :, base + 512:base + half],
                                 start=True, stop=True, perf_mode=DR)
                slot = k * 4 + p * 2 + ch
                col = slot // 2
                if slot % 2 == 0:
                    scr = scrp.tile([128, half], BF16, tag="scr")
                    nc.scalar.activation(out=scr[:], in_=ps[:, 0:half],
                                         func=ACTF.Abs,
                                         accum_out=sa[:, col:col + 1])
                else:
                    nc.vector.tensor_reduce(
                        out=sd[:, col:col + 1], in_=ps[:, 0:half],
                        axis=mybir.AxisListType.X, op=ALU.add,
                        apply_absolute_value=True)


_PROGRAMS = {}


def _get_program(d_chunks=None, nb=None, n_reps=1):
    if d_chunks is None or nb is None:
        assert _LAYOUT is not None, "call kernel() first"
        d_chunks, _, nb = _LAYOUT
    key = (d_chunks, nb, n_reps)
    if key not in _PROGRAMS:
        _PROGRAMS[key] = _build_program(d_chunks, nb, n_reps)
    return _PROGRAMS[key]


def _build_program_reps(n_reps):
    assert _LAYOUT is not None, "call kernel() (or _prep_core) first"
    d, _, nb = _LAYOUT
    return _get_program(d, nb, n_reps)


def _core_box_count(in_len, out_len):
    return int(np.sum(out_len.astype(np.int64) * (TIN - in_len.astype(np.int64))))


def _core_band_cols(out_len):
    return -(-int(np.sum(out_len.astype(np.int64))) * BW // 128)


def _prep_core(al, melo, melp_, melt, go, gt, in_len, out_len):
    """Build one core's input map. al: [BPC, TOUT, TIN] etc. (numpy f32)."""
    global _LAYOUT
    in_len = np.asarray(in_len, dtype=np.int64)
    out_len = np.asarray(out_len, dtype=np.int64)
    if _LAYOUT is None:
        # standalone use: size from this core with margin
        _LAYOUT = _canvas_layout(int(_core_box_count(in_len, out_len) * 1.25),
                                 _core_band_cols(out_len) + 64)
    d, ranges, nb = _LAYOUT

    # mel: [128, tile k][mo | mt | mp] fp8
    m3 = np.stack([melo.reshape(MROWS, TOUT),
                   melt.reshape(MROWS, TOUT),
                   melp_.reshape(MROWS, TOUT)], axis=1)     # [640, 3, 2000]
    m3 = (m3.reshape(NMT, 128, 3, TOUT).transpose(1, 0, 2, 3)
          .reshape(128, NMT * 3 * TOUT))
    mel8 = np.ascontiguousarray(m3).astype(F8NP)

    # canvas groups
    box_vals = [al[s, :out_len[s], in_len[s]:] for s in range(BPC)]
    box = (np.concatenate([v.ravel() for v in box_vals])
           if box_vals else np.zeros(0, np.float32))
    dirv = np.concatenate([al[s, :IMID + 1, :IMID][_DIR_MASK]
                           for s in range(BPC)])
    tailv = np.concatenate([al[s, IMID + 1:TIN, :][_TAIL_MASK]
                            for s in range(BPC)])

    cv = np.zeros((d, 512, 128), np.float32)
    for vals, (a, b) in zip((box, dirv, tailv), ranges):
        cap = d * (b - a) * 128
        assert len(vals) <= cap, f"canvas overflow: {len(vals)} > {cap}"
        pad = np.zeros(cap, np.float32)
        pad[:len(vals)] = vals * ASCALE
        cv[:, a:b, :] = pad.reshape(d, b - a, 128)
    cv8 = np.ascontiguousarray(cv.transpose(2, 0, 1).reshape(128, d * 512)
                               ).astype(F8NP)

    # gaussian band: 4 columns around j* = i*in/out for valid rows
    bands = []
    bws = []
    for s in range(BPC):
        ol, il = int(out_len[s]), int(in_len[s])
        iv = np.arange(ol, dtype=np.float64)
        jstar = iv * il / ol
        s0 = np.clip(np.floor(jstar).astype(np.int64) - 1, 0, TIN - BW)
        jb = s0[:, None] + np.arange(BW)[None, :]            # [ol, BW]
        bands.append(al[s, iv.astype(np.int64)[:, None], jb].ravel())
        dlt = iv[:, None] - jb * (float(ol) / il)
        w = np.exp(ESCALE * dlt * dlt)
        w[jb >= il] = 0.0
        bws.append(w.ravel())
    bflat = np.concatenate(bands)
    wflat = np.concatenate(bws)
    bpad = np.zeros(128 * nb, np.float32)
    bpad[:len(bflat)] = bflat
    wpad = np.zeros(128 * nb, np.float32)
    wpad[:len(wflat)] = wflat

    # identity stationary: [p, 0*128+m]=+1[p==m], [p, 128+m]=-1[p==m]
    idw = np.zeros((128, 256), np.float32)
    idw[np.arange(128), np.arange(128)] = 1.0
    idw[np.arange(128), 128 + np.arange(128)] = -1.0

    return {
        "idw": idw.astype(F8NP),
        "mel": mel8,
        "cv": cv8,
        "band": bpad.reshape(128, nb).astype(BFNP),
        "bw": wpad.reshape(128, nb).astype(BFNP),
        "gate": np.ascontiguousarray(
            np.concatenate([go.reshape(128, GCOLS), gt.reshape(128, GCOLS)],
                           axis=1), dtype=np.float32),
    }


def kernel(mel_out, mel_out_postnet, gate_out, alignments,
           mel_target, gate_target, input_lengths, output_lengths,
           _results_hook=None):
    global _LAYOUT
    mel_out = np.asarray(mel_out, dtype=np.float32)
    mel_out_postnet = np.asarray(mel_out_postnet, dtype=np.float32)
    gate_out = np.asarray(gate_out, dtype=np.float32)
    alignments = np.asarray(alignments, dtype=np.float32)
    mel_target = np.asarray(mel_target, dtype=np.float32)
    gate_target = np.asarray(gate_target, dtype=np.float32)
    in_len = np.asarray(input_lengths).astype(np.int64)
    out_len = np.asarray(output_lengths).astype(np.int64)

    # global layout from all cores (one SPMD program)
    max_box = 0
    max_nb = 0
    for c in range(NCORES):
        sl = slice(BPC * c, BPC * (c + 1))
        max_box = max(max_box, _core_box_count(in_len[sl], out_len[sl]))
        max_nb = max(max_nb, _core_band_cols(out_len[sl]))
    lay = _canvas_layout(max_box, max_nb)
    if _LAYOUT is None or _LAYOUT[0] < lay[0] or _LAYOUT[2] < lay[2]:
        _LAYOUT = lay
    d, ranges, nb = _LAYOUT

    in_maps = []
    for c in range(NCORES):
        sl = slice(BPC * c, BPC * (c + 1))
        in_maps.append(_prep_core(
            alignments[sl], mel_out[sl], mel_out_postnet[sl], mel_target[sl],
            gate_out[sl], gate_target[sl], in_len[sl], out_len[sl]))

    nc = _get_program(d, nb)
    res = run_bass_kernel_spmd(nc, in_maps, core_ids=list(range(NCORES)))
    if _results_hook is not None:
        _results_hook(res)

    mel_sum = gsp = grelu = gxz = gauss = 0.0
    att = box = 0.0
    (ba, bb), (da, db), (ta, tb) = ranges
    for c in range(NCORES):
        out = res.results[c]
        sa = out["sa"].astype(np.float64)
        sd = out["sd"].astype(np.float64)
        cs = out["cs"].astype(np.float64)[0]

        mel_sum += sa[:, 0:10].sum() + sd[:, 0:10].sum()
        gsp += sa[:, 10].sum()
        grelu += sa[:, 11].sum()
        gxz += sd[:, 10].sum()
        gauss += sd[:, 11].sum()

        box_tail = cs[ba:bb].sum() / ASCALE
        att_dir = cs[da:db].sum() / ASCALE
        att_tail = cs[ta:tb].sum() / ASCALE

        sl = slice(BPC * c, BPC * (c + 1))
        att += BPC * ATT_CONST + att_dir - att_tail
        box += float(out_len[sl].sum()) - box_tail

    n_mel = B * MEL * TOUT
    n_gate = B * TOUT
    mel_loss = mel_sum / n_mel
    gate_loss = (grelu - gxz + gsp) / n_gate
    att_loss = att / B
    ga_loss = (box - gauss) / B
    total = (MEL_W * mel_loss + GATE_W * gate_loss
             + ATT_W * att_loss + GA_W * ga_loss)
    f = np.float32
    return (f(total), f(mel_loss), f(gate_loss), f(att_loss), f(ga_loss))


# revision 18
# speedup vs baseline: 2.7384x; 1.1023x over previous
"""Trainium2 Bass kernel for the combined Tacotron-style loss.

Strategy (pure data parallel, 8 samples per core on 8 NeuronCores).

Every loss term is a big reduction, so the kernel is built around moving as
few HBM bytes as possible and reducing them on the widest engines:

  - mel L1 terms: mo/mt/mp stream in fp8 (statistically safe for a 10M-element
    mean at 2e-2 tol).  The PE computes (mo-mt) and (mt-mp) with a +I/-I
    DoubleRow fp8 matmul into PSUM f32; ACT (Abs + accum) and DVE
    (tensor_reduce abs-add) split the row-sum work.
  - attention / guided-attention box terms: alignment rows are normalized
    (sum_j A[i,j] == 1), so sums over wide row prefixes are computed as
    1 - (narrow tail sum).  The host packs exactly the needed tail/window
    elements into a [128, D*512] fp8 "canvas"; a ones-stationary DoubleRow
    matmul chain column-sums it into one PSUM bank.  Column index mod 512
    identifies the group (box-tail / att-direct / att-tail) on the host.
  - gaussian term: sigma=0.4 makes exp(-(i-j*out/in)^2/(2s^2)) a <=4-column
    band; host gathers band values + weights, one DVE mult+accum reduces it.
  - gate BCE: f32, ACT Abs/Exp/Ln + Relu + DVE x*z, all with fused accum.

Host combines all partial sums in float64.
"""

import ml_dtypes
import numpy as np

import concourse.bacc as bacc
import concourse.mybir as mybir
from concourse import bass
from concourse.bass_utils import run_bass_kernel_spmd
from concourse.tile import TileContext

F32 = mybir.dt.float32
BF16 = mybir.dt.bfloat16
F8 = mybir.dt.float8e4
ALU = mybir.AluOpType
ACTF = mybir.ActivationFunctionType
DR = mybir.MatmulPerfMode.DoubleRow

F8NP = ml_dtypes.float8_e4m3
BFNP = ml_dtypes.bfloat16

# Problem shapes (hardcoded per contract).
B, MEL, TOUT, TIN = 64, 80, 2000, 400
NCORES = 8
BPC = B // NCORES                  # samples per core
MROWS = BPC * MEL                  # 640 mel rows per core
NMT = MROWS // 128                 # 5 mel row-tiles
GCOLS = BPC * TOUT // 128          # 125 gate cols ([128, 125] layout)
BW = 4                             # gaussian band width
SIGMA = 0.4
ESCALE = -1.0 / (2.0 * SIGMA * SIGMA)
MEL_W, GATE_W, ATT_W, GA_W = 1.0, 1.0, 0.1, 0.1
ASCALE = 16384.0                   # 2**14: puts fp8 alignment values in normal range

IMID = TIN // 2                    # 200: att rows i<=IMID summed directly,
#                                    i>IMID via 1 - tail
N_DIR = BPC * (IMID * (IMID + 1) // 2)          # direct window elements/core
N_ATT_TAIL = BPC * ((IMID - 1) * IMID // 2)     # att tail elements/core
ATT_CONST = (TOUT - TIN) + (TIN - 1 - IMID)     # exact-1.0 rows per sample

# att-direct mask: rows i=0..IMID, cols j<i  (j <= IMID-1)
_DIR_MASK = np.arange(IMID)[None, :] < np.arange(IMID + 1)[:, None]
# att-tail mask: rows i=IMID+1..TIN-1, cols j>=i
_TAIL_MASK = (np.arange(TIN)[None, :]
              >= (IMID + 1 + np.arange(TIN - 1 - IMID))[:, None])

# mel chunk-read engine assignment: alternate ACT / DVE (GPSIMD cannot
# read PSUM, so it instead takes all the small SBUF-side reductions)
N_MEL_SLOTS = NMT * 4              # 5 tiles x 2 pairs x 2 chunks
READER = ['A', 'D'] * 10
SA_COLS = 16                       # ACT stats: 0..9 mel, 10 softplus
SD_COLS = 16                       # DVE stats: 0..9 mel
SP_COLS = 8                        # Pool stats: 0 x*z, 1 relu, 2 band

# Canvas layout: set lazily from the actual inputs (sizes depend on
# input/output lengths).  (n_chunks D, (a,b) col ranges per group, band cols)
_LAYOUT = None


def _canvas_layout(max_box, nb_cols):
    """Pick D (512-col canvas chunks) + column ranges for the 3 groups."""
    sizes = [max_box, N_DIR, N_ATT_TAIL]
    total = sum(sizes)
    d = max(2, -(-total // (128 * 512)))
    while True:
        cols = [-(-s // (128 * d)) for s in sizes]
        if sum(cols) <= 512:
            break
        d += 1
    d = -(-d // 2) * 2             # even: canvas is consumed in chunk pairs
    ranges = []
    a = 0
    for c in cols:
        ranges.append((a, a + c))
        a += c
    nb = -(-nb_cols // 64) * 64
    return (d, tuple(ranges), nb)


def _build_program(d_chunks, nb, n_reps=1):
    nc = bacc.Bacc(
        "TRN2",
        target_bir_lowering=False,
        debug=False,
        enable_asserts=False,
        num_devices=NCORES,
    )

    # one packed byte tensor for all the small inputs:
    # [id 256B | gate f32 1000B | band bf16 2*nb | bw bf16 2*nb]
    auxw = 256 + 8 * GCOLS + 4 * nb
    d_aux = nc.dram_tensor("aux", (128, auxw), mybir.dt.uint8,
                           kind="ExternalInput").ap()
    d_mel = nc.dram_tensor("mel", (128, NMT * 3 * TOUT), F8,
                           kind="ExternalInput").ap()
    d_cv = nc.dram_tensor("cv", (128, d_chunks * 512), F8,
                          kind="ExternalInput").ap()

    o_sa = nc.dram_tensor("sa", (128, SA_COLS), F32, kind="ExternalOutput").ap()
    o_sd = nc.dram_tensor("sd", (128, SD_COLS), F32, kind="ExternalOutput").ap()
    o_sp = nc.dram_tensor("sp", (128, SP_COLS), F32, kind="ExternalOutput").ap()
    o_cs = nc.dram_tensor("cs", (1, 512), F32, kind="ExternalOutput").ap()

    with TileContext(nc) as tc:
        with (
            tc.tile_pool(name="small", bufs=1) as sp,
            tc.tile_pool(name="cvp", bufs=3) as cvp,
            tc.tile_pool(name="melp", bufs=3) as melp,
            tc.tile_pool(name="scrp", bufs=2) as scrp,
            tc.tile_pool(name="pscrp", bufs=2) as pscrp,
            tc.tile_pool(name="pscs", bufs=1, space="PSUM") as pscs,
            tc.tile_pool(name="psmel", bufs=3, space="PSUM") as psmel,
        ):
            aux_sb = sp.tile([128, 256 + 8 * GCOLS + 4 * nb], mybir.dt.uint8)
            nc.sync.dma_start(out=aux_sb[:], in_=d_aux)
            id_sb = aux_sb[:, 0:256].bitcast(F8)
            gate_sb = aux_sb[:, 256:256 + 8 * GCOLS].bitcast(F32)
            b0 = 256 + 8 * GCOLS
            band_sb = aux_sb[:, b0:b0 + 2 * nb].bitcast(BF16)
            bw_sb = aux_sb[:, b0 + 2 * nb:b0 + 4 * nb].bitcast(BF16)

            # ones stationary for canvas colsums: DoubleRow requires the
            # k-pair dim stride to be a multiple of 16
            ones2 = sp.tile([128, 32], F8)
            nc.gpsimd.memset(ones2[:], 1.0)
            sa = sp.tile([128, SA_COLS], F32)
            nc.vector.memset(sa[:], 0.0)
            sd = sp.tile([128, SD_COLS], F32)
            nc.vector.memset(sd[:], 0.0)
            spst = sp.tile([128, SP_COLS], F32)
            nc.gpsimd.memset(spst[:], 0.0)

            cs_ps = pscs.tile([1, 512], F32)

            for _rep in range(n_reps):
                _emit_body(nc, sp, cvp, melp, scrp, pscrp, psmel,
                           id_sb, gate_sb, band_sb, bw_sb, ones2,
                           sa, sd, spst, cs_ps, d_cv, d_mel, d_chunks)

            cs_sb = sp.tile([1, 512], F32)
            nc.vector.tensor_copy(out=cs_sb[:], in_=cs_ps[:])
            nc.sync.dma_start(out=o_cs, in_=cs_sb[:])
            nc.sync.dma_start(out=o_sa, in_=sa[:])
            nc.sync.dma_start(out=o_sd, in_=sd[:])
            nc.sync.dma_start(out=o_sp, in_=spst[:])

    nc.compile()
    return nc


def _emit_body(nc, sp, cvp, melp, scrp, pscrp, psmel,
               id_sb, gate_sb, band_sb, bw_sb, ones2,
               sa, sd, spst, cs_ps, d_cv, d_mel, d_chunks):
    nb = band_sb.shape[1]

    # --- gate BCE cheap parts + gaussian band, all on GPSIMD (SBUF only) ---
    go = gate_sb[:, 0:GCOLS]
    gt = gate_sb[:, GCOLS:2 * GCOLS]
    g1 = sp.tile([128, GCOLS], F32, tag="g1")
    nc.scalar.activation(out=g1[:], in_=go, func=ACTF.Abs)
    g5 = sp.tile([128, GCOLS], F32, tag="g5")
    nc.vector.scalar_tensor_tensor(
        out=g5[:], in0=go, scalar=0.0, in1=gt,
        op0=ALU.add, op1=ALU.mult, accum_out=spst[:, 0:1])
    g6 = sp.tile([128, GCOLS], F32, tag="g6")
    nc.vector.scalar_tensor_tensor(
        out=g6[:], in0=go, scalar=0.0, in1=go,
        op0=ALU.is_gt, op1=ALU.mult, accum_out=spst[:, 1:2])
    bscr = sp.tile([128, nb], BF16, tag="bscr")
    nc.vector.scalar_tensor_tensor(
        out=bscr[:], in0=band_sb, scalar=1.0, in1=bw_sb,
        op0=ALU.mult, op1=ALU.mult, accum_out=spst[:, 2:3])

    ones_v = ones2[:].rearrange("p (two s) -> p two s", two=2)[:, :, 0:1]
    id2 = id_sb.rearrange("p (two m) -> p two m", two=2)

    # --- mel L1 (PE diffs -> ACT/DVE abs+row-sum), canvas colsum chain
    #     interleaved so mel readers start as early as possible ---
    half = TOUT // 2
    n_pairs = d_chunks // 2
    total_cv = d_chunks * 512
    cv_off = [0]
    cv_pair = [0]

    def emit_canvas_dma():
        if cv_off[0] >= total_cv:
            return
        w = min(2048, total_cv - cv_off[0])
        cvt = cvp.tile([128, 2048], F8, tag="cv")
        nc.sync.dma_start(out=cvt[:, 0:w], in_=d_cv[:, cv_off[0]:cv_off[0] + w])
        for h in range(w // 1024):
            nc.tensor.matmul(
                cs_ps[:], ones_v,
                cvt[:, h * 1024:(h + 1) * 1024].rearrange(
                    "p (two j) -> p two j", two=2),
                start=(cv_pair[0] == 0),
                stop=(cv_pair[0] == n_pairs - 1),
                perf_mode=DR,
                skip_group_check=True,
            )
            cv_pair[0] += 1
        cv_off[0] += w

    n_cv_dmas = (total_cv + 2047) // 2048
    # canvas DMA schedule: spread between mel tiles (mel tile 0 first)
    cv_after = {0: 2, 1: 2, 2: 2, 3: 1, 4: 0}
    rem = n_cv_dmas - sum(cv_after.values())
    cv_after[4] += max(rem, 0)

    ncols = {'A': 0, 'D': 0}
    for k in range(NMT):
        mt = melp.tile([128, 3 * TOUT], F8, tag="mel")
        nc.sync.dma_start(out=mt[:], in_=d_mel[:, k * 3 * TOUT:(k + 1) * 3 * TOUT])
        for p in range(2):
            # pair 0: planes (mo, mt) -> mo - mt; pair 1: (mt, mp) -> mt - mp
            pv = mt[:, p * TOUT:p * TOUT + 2 * TOUT].rearrange(
                "p (two j) -> p two j", two=2)
            for ch in range(2):
                ps = psmel.tile([128, 1024], F32, tag="mps")
                base = ch * half
                nc.tensor.matmul(ps[:, 0:512], id2, pv[:, :, base:base + 512],
                                 start=True, stop=True, perf_mode=DR,
                                 skip_group_check=True)
                nc.tensor.matmul(ps[:, 512:half], id2,
                                 pv[:, :, base + 512:base + half],
                                 start=True, stop=True, perf_mode=DR,
                                 skip_group_check=True)
                eng = READER[k * 4 + p * 2 + ch]
                col = ncols[eng]
                ncols[eng] += 1
                if eng == 'A':
                    scr = scrp.tile([128, half], BF16, tag="scr")
                    nc.scalar.activation(out=scr[:], in_=ps[:, 0:half],
                                         func=ACTF.Abs,
                                         accum_out=sa[:, col:col + 1])
                else:
                    nc.vector.tensor_reduce(
                        out=sd[:, col:col + 1], in_=ps[:, 0:half],
                        axis=mybir.AxisListType.X, op=ALU.add,
                        apply_absolute_value=True)
        for _ in range(cv_after[k]):
            emit_canvas_dma()
    while cv_off[0] < total_cv:
        emit_canvas_dma()

    # --- gate BCE softplus chain (deferred: its ACT table load lands in
    #     the tail while DVE/Pool drain their last mel reads) ---
    g2 = sp.tile([128, GCOLS], F32, tag="g2")
    nc.scalar.activation(out=g2[:], in_=g1[:], func=ACTF.Exp, scale=-1.0)
    g3 = sp.tile([128, GCOLS], F32, tag="g3")
    nc.scalar.activation(out=g3[:], in_=g2[:], func=ACTF.Ln, bias=1.0,
                         accum_out=sa[:, 10:11])


_PROGRAMS = {}


def _get_program(d_chunks=None, nb=None, n_reps=1):
    if d_chunks is None or nb is None:
        assert _LAYOUT is not None, "call kernel() first"
        d_chunks, _, nb = _LAYOUT
    key = (d_chunks, nb, n_reps)
    if key not in _PROGRAMS:
        _PROGRAMS[key] = _build_program(d_chunks, nb, n_reps)
    return _PROGRAMS[key]


def _build_program_reps(n_reps):
    assert _LAYOUT is not None, "call kernel() (or _prep_core) first"
    d, _, nb = _LAYOUT
    return _get_program(d, nb, n_reps)


def _core_box_count(in_len, out_len):
    return int(np.sum(out_len.astype(np.int64) * (TIN - in_len.astype(np.int64))))


def _core_band_cols(out_len):
    return -(-int(np.sum(out_len.astype(np.int64))) * BW // 128)


def _prep_core(al, melo, melp_, melt, go, gt, in_len, out_len):
    """Build one core's input map. al: [BPC, TOUT, TIN] etc. (numpy f32)."""
    global _LAYOUT
    in_len = np.asarray(in_len, dtype=np.int64)
    out_len = np.asarray(out_len, dtype=np.int64)
    if _LAYOUT is None:
        # standalone use: size from this core with margin
        _LAYOUT = _canvas_layout(int(_core_box_count(in_len, out_len) * 1.25),
                                 _core_band_cols(out_len) + 64)
    d, ranges, nb = _LAYOUT

    # mel: [128, tile k][mo | mt | mp] fp8
    m3 = np.stack([melo.reshape(MROWS, TOUT),
                   melt.reshape(MROWS, TOUT),
                   melp_.reshape(MROWS, TOUT)], axis=1)     # [640, 3, 2000]
    m3 = (m3.reshape(NMT, 128, 3, TOUT).transpose(1, 0, 2, 3)
          .reshape(128, NMT * 3 * TOUT))
    mel8 = np.ascontiguousarray(m3).astype(F8NP)

    # canvas groups
    box_vals = [al[s, :out_len[s], in_len[s]:] for s in range(BPC)]
    box = (np.concatenate([v.ravel() for v in box_vals])
           if box_vals else np.zeros(0, np.float32))
    dirv = np.concatenate([al[s, :IMID + 1, :IMID][_DIR_MASK]
                           for s in range(BPC)])
    tailv = np.concatenate([al[s, IMID + 1:TIN, :][_TAIL_MASK]
                            for s in range(BPC)])

    cv = np.zeros((d, 512, 128), np.float32)
    for vals, (a, b) in zip((box, dirv, tailv), ranges):
        cap = d * (b - a) * 128
        assert len(vals) <= cap, f"canvas overflow: {len(vals)} > {cap}"
        pad = np.zeros(cap, np.float32)
        pad[:len(vals)] = vals * ASCALE
        cv[:, a:b, :] = pad.reshape(d, b - a, 128)
    cv8 = np.ascontiguousarray(cv.transpose(2, 0, 1).reshape(128, d * 512)
                               ).astype(F8NP)

    # gaussian band: 4 columns around j* = i*in/out for valid rows
    bands = []
    bws = []
    for s in range(BPC):
        ol, il = int(out_len[s]), int(in_len[s])
        iv = np.arange(ol, dtype=np.float64)
        jstar = iv * il / ol
        s0 = np.clip(np.floor(jstar).astype(np.int64) - 1, 0, TIN - BW)
        jb = s0[:, None] + np.arange(BW)[None, :]            # [ol, BW]
        bands.append(al[s, iv.astype(np.int64)[:, None], jb].ravel())
        dlt = iv[:, None] - jb * (float(ol) / il)
        w = np.exp(ESCALE * dlt * dlt)
        w[jb >= il] = 0.0
        bws.append(w.ravel())
    bflat = np.concatenate(bands)
    wflat = np.concatenate(bws)
    bpad = np.zeros(128 * nb, np.float32)
    bpad[:len(bflat)] = bflat
    wpad = np.zeros(128 * nb, np.float32)
    wpad[:len(wflat)] = wflat

    # identity stationary: [p, 0*128+m]=+1[p==m], [p, 128+m]=-1[p==m]
    idw = np.zeros((128, 256), np.float32)
    idw[np.arange(128), np.arange(128)] = 1.0
    idw[np.arange(128), 128 + np.arange(128)] = -1.0

    gate = np.ascontiguousarray(
        np.concatenate([go.reshape(128, GCOLS), gt.reshape(128, GCOLS)],
                       axis=1), dtype=np.float32)
    u8 = np.uint8
    aux = np.concatenate([
        np.ascontiguousarray(idw.astype(F8NP)).view(u8),
        gate.view(u8),
        np.ascontiguousarray(bpad.reshape(128, nb).astype(BFNP)).view(u8),
        np.ascontiguousarray(wpad.reshape(128, nb).astype(BFNP)).view(u8),
    ], axis=1)

    return {"aux": np.ascontiguousarray(aux), "mel": mel8, "cv": cv8}


def kernel(mel_out, mel_out_postnet, gate_out, alignments,
           mel_target, gate_target, input_lengths, output_lengths,
           _results_hook=None):
    global _LAYOUT
    mel_out = np.asarray(mel_out, dtype=np.float32)
    mel_out_postnet = np.asarray(mel_out_postnet, dtype=np.float32)
    gate_out = np.asarray(gate_out, dtype=np.float32)
    alignments = np.asarray(alignments, dtype=np.float32)
    mel_target = np.asarray(mel_target, dtype=np.float32)
    gate_target = np.asarray(gate_target, dtype=np.float32)
    in_len = np.asarray(input_lengths).astype(np.int64)
    out_len = np.asarray(output_lengths).astype(np.int64)

    # global layout from all cores (one SPMD program)
    max_box = 0
    max_nb = 0
    for c in range(NCORES):
        sl = slice(BPC * c, BPC * (c + 1))
        max_box = max(max_box, _core_box_count(in_len[sl], out_len[sl]))
        max_nb = max(max_nb, _core_band_cols(out_len[sl]))
    lay = _canvas_layout(max_box, max_nb)
    if _LAYOUT is None or _LAYOUT[0] < lay[0] or _LAYOUT[2] < lay[2]:
        _LAYOUT = lay
    d, ranges, nb = _LAYOUT

    in_maps = []
    for c in range(NCORES):
        sl = slice(BPC * c, BPC * (c + 1))
        in_maps.append(_prep_core(
            alignments[sl], mel_out[sl], mel_out_postnet[sl], mel_target[sl],
            gate_out[sl], gate_target[sl], in_len[sl], out_len[sl]))

    nc = _get_program(d, nb)
    res = run_bass_kernel_spmd(nc, in_maps, core_ids=list(range(NCORES)))
    if _results_hook is not None:
        _results_hook(res)

    mel_sum = gsp = grelu = gxz = gauss = 0.0
    att = box = 0.0
    (ba, bb), (da, db), (ta, tb) = ranges
    for c in range(NCORES):
        out = res.results[c]
        sa = out["sa"].astype(np.float64)
        sd = out["sd"].astype(np.float64)
        spst = out["sp"].astype(np.float64)
        cs = out["cs"].astype(np.float64)[0]

        mel_sum += sa[:, 0:10].sum() + sd[:, 0:10].sum()
        gsp += sa[:, 10].sum()
        gxz += spst[:, 0].sum()
        grelu += spst[:, 1].sum()
        gauss += spst[:, 2].sum()

        box_tail = cs[ba:bb].sum() / ASCALE
        att_dir = cs[da:db].sum() / ASCALE
        att_tail = cs[ta:tb].sum() / ASCALE

        sl = slice(BPC * c, BPC * (c + 1))
        att += BPC * ATT_CONST + att_dir - att_tail
        box += float(out_len[sl].sum()) - box_tail

    n_mel = B * MEL * TOUT
    n_gate = B * TOUT
    mel_loss = mel_sum / n_mel
    gate_loss = (grelu - gxz + gsp) / n_gate
    att_loss = att / B
    ga_loss = (box - gauss) / B
    total = (MEL_W * mel_loss + GATE_W * gate_loss
             + ATT_W * att_loss + GA_W * ga_loss)
    f = np.float32
    return (f(total), f(mel_loss), f(gate_loss), f(att_loss), f(ga_loss))


# revision 29
# speedup vs baseline: 2.8279x; 1.0327x over previous
"""Trainium2 Bass kernel for the combined Tacotron-style loss.

Strategy (pure data parallel, 8 samples per core on 8 NeuronCores).

Every loss term is a big reduction, so the kernel is built around moving as
few HBM bytes as possible and reducing them on the widest engines:

  - mel L1 terms: mo/mt/mp stream in fp8 (statistically safe for a 10M-element
    mean at 2e-2 tol).  The PE computes (mo-mt) and (mt-mp) with a +I/-I
    DoubleRow fp8 matmul into PSUM f32; ACT (Abs + accum) and DVE
    (tensor_reduce abs-add) split the row-sum work.
  - attention / guided-attention box terms: alignment rows are normalized
    (sum_j A[i,j] == 1), so sums over wide row prefixes are computed as
    1 - (narrow tail sum).  The host packs exactly the needed tail/window
    elements into a [128, D*512] fp8 "canvas"; a ones-stationary DoubleRow
    matmul chain column-sums it into one PSUM bank.  Column index mod 512
    identifies the group (box-tail / att-direct / att-tail) on the host.
  - gaussian term: sigma=0.4 makes exp(-(i-j*out/in)^2/(2s^2)) a <=4-column
    band; host gathers band values + weights, one DVE mult+accum reduces it.
  - gate BCE: f32, ACT Abs/Exp/Ln + Relu + DVE x*z, all with fused accum.

Host combines all partial sums in float64.
"""

import ml_dtypes
import numpy as np

import concourse.bacc as bacc
import concourse.mybir as mybir
from concourse import bass
from concourse.bass_utils import run_bass_kernel_spmd
from concourse.tile import TileContext

F32 = mybir.dt.float32
BF16 = mybir.dt.bfloat16
F8 = mybir.dt.float8e4
ALU = mybir.AluOpType
ACTF = mybir.ActivationFunctionType
DR = mybir.MatmulPerfMode.DoubleRow

F8NP = ml_dtypes.float8_e4m3
BFNP = ml_dtypes.bfloat16

# Problem shapes (hardcoded per contract).
B, MEL, TOUT, TIN = 64, 80, 2000, 400
NCORES = 8
BPC = B // NCORES                  # samples per core
MROWS = BPC * MEL                  # 640 mel rows per core
NMT = MROWS // 128                 # 5 mel row-tiles
GCOLS = BPC * TOUT // 128          # 125 gate cols ([128, 125] layout)
BW = 4                             # gaussian band width
SIGMA = 0.4
ESCALE = -1.0 / (2.0 * SIGMA * SIGMA)
MEL_W, GATE_W, ATT_W, GA_W = 1.0, 1.0, 0.1, 0.1
ASCALE = 16384.0                   # 2**14: puts fp8 alignment values in normal range

IMID = TIN // 2                    # 200: att rows i<=IMID summed directly,
#                                    i>IMID via 1 - tail
N_DIR = BPC * (IMID * (IMID + 1) // 2)          # direct window elements/core
N_ATT_TAIL = BPC * ((IMID - 1) * IMID // 2)     # att tail elements/core
ATT_CONST = (TOUT - TIN) + (TIN - 1 - IMID)     # exact-1.0 rows per sample

# att-direct mask: rows i=0..IMID, cols j<i  (j <= IMID-1)
_DIR_MASK = np.arange(IMID)[None, :] < np.arange(IMID + 1)[:, None]
# att-tail mask: rows i=IMID+1..TIN-1, cols j>=i
_TAIL_MASK = (np.arange(TIN)[None, :]
              >= (IMID + 1 + np.arange(TIN - 1 - IMID))[:, None])

# mel chunk-read engine assignment: alternate ACT / DVE (GPSIMD cannot
# read PSUM, so it instead takes all the small SBUF-side reductions)
N_MEL_SLOTS = NMT * 4              # 5 tiles x 2 halves x 2 pairs
READER = ['A', 'D'] * 10
MELH = TOUT // 2                   # 1000 data cols per mel half-tile
MELS = 1008                        # padded plane stride (DoubleRow: %16 == 0)
MELW = 3 * MELS                    # half-tile width (mo | mt | mp planes)
SA_COLS = 16                       # ACT stats: 0..9 mel, 10 softplus
SD_COLS = 16                       # DVE stats: 0..9 mel
SP_COLS = 8                        # Pool stats: 0 x*z, 1 relu, 2 band

# Canvas layout: set lazily from the actual inputs (sizes depend on
# input/output lengths).  (n_chunks D, (a,b) col ranges per group, band cols)
_LAYOUT = None


def _canvas_layout(max_box, nb_cols):
    """Pick D (512-col canvas chunks) + column ranges for the 3 groups."""
    sizes = [max_box, N_DIR, N_ATT_TAIL]
    total = sum(sizes)
    d = max(2, -(-total // (128 * 512)))
    while True:
        cols = [-(-s // (128 * d)) for s in sizes]
        if sum(cols) <= 512:
            break
        d += 1
    ranges = []
    a = 0
    for c in cols:
        ranges.append((a, a + c))
        a += c
    nb = -(-nb_cols // 64) * 64
    return (d, tuple(ranges), nb)


def _build_program(d_chunks, nb, n_reps=1):
    nc = bacc.Bacc(
        "TRN2",
        target_bir_lowering=False,
        debug=False,
        enable_asserts=False,
        num_devices=NCORES,
    )

    # one packed byte tensor for all the small inputs:
    # [id 256B | gate f32 1000B | band bf16 2*nb | bw bf16 2*nb]
    auxw = 256 + 8 * GCOLS + 4 * nb
    d_aux = nc.dram_tensor("aux", (128, auxw), mybir.dt.uint8,
                           kind="ExternalInput").ap()
    d_mel = nc.dram_tensor("mel", (128, NMT * 2 * MELW), F8,
                           kind="ExternalInput").ap()
    d_cv = nc.dram_tensor("cv", (128, d_chunks * 512), F8,
                          kind="ExternalInput").ap()

    o_sa = nc.dram_tensor("sa", (128, SA_COLS), F32, kind="ExternalOutput").ap()
    o_sd = nc.dram_tensor("sd", (128, SD_COLS), F32, kind="ExternalOutput").ap()
    o_sp = nc.dram_tensor("sp", (128, SP_COLS), F32, kind="ExternalOutput").ap()
    o_cs = nc.dram_tensor("cs", (1, 512), F32, kind="ExternalOutput").ap()

    with TileContext(nc) as tc:
        with (
            tc.tile_pool(name="small", bufs=1) as sp,
            tc.tile_pool(name="cvp", bufs=3) as cvp,
            tc.tile_pool(name="melp", bufs=3) as melp,
            tc.tile_pool(name="scrp", bufs=2) as scrp,
            tc.tile_pool(name="pscrp", bufs=2) as pscrp,
            tc.tile_pool(name="pscs", bufs=1, space="PSUM") as pscs,
            tc.tile_pool(name="psmel", bufs=3, space="PSUM") as psmel,
        ):
            aux_sb = sp.tile([128, 256 + 8 * GCOLS + 4 * nb], mybir.dt.uint8)
            nc.sync.dma_start(out=aux_sb[:], in_=d_aux)
            id_sb = aux_sb[:, 0:256].bitcast(F8)
            gate_sb = aux_sb[:, 256:256 + 8 * GCOLS].bitcast(F32)
            b0 = 256 + 8 * GCOLS
            band_sb = aux_sb[:, b0:b0 + 2 * nb].bitcast(BF16)
            bw_sb = aux_sb[:, b0 + 2 * nb:b0 + 4 * nb].bitcast(BF16)

            # ones stationary for canvas colsums: DoubleRow requires the
            # k-pair dim stride to be a multiple of 16
            ones2 = sp.tile([128, 32], F8)
            nc.gpsimd.memset(ones2[:], 1.0)
            sa = sp.tile([128, SA_COLS], F32)
            nc.vector.memset(sa[:], 0.0)
            sd = sp.tile([128, SD_COLS], F32)
            nc.vector.memset(sd[:], 0.0)
            spst = sp.tile([128, SP_COLS], F32)
            nc.gpsimd.memset(spst[:], 0.0)

            cs_ps = pscs.tile([1, 512], F32)

            for _rep in range(n_reps):
                _emit_body(nc, sp, cvp, melp, scrp, pscrp, psmel,
                           id_sb, gate_sb, band_sb, bw_sb, ones2,
                           sa, sd, spst, cs_ps, d_cv, d_mel, d_chunks)

            # spread the output DMAs across queues so their issue/sem
            # latencies overlap instead of stacking on one SEQ
            cs_sb = sp.tile([1, 512], F32)
            nc.vector.tensor_copy(out=cs_sb[:], in_=cs_ps[:])
            nc.gpsimd.dma_start(out=o_cs, in_=cs_sb[:])
            nc.scalar.dma_start(out=o_sa, in_=sa[:])
            nc.sync.dma_start(out=o_sd, in_=sd[:])
            nc.gpsimd.dma_start(out=o_sp, in_=spst[:])

    nc.compile()
    return nc


def _emit_body(nc, sp, cvp, melp, scrp, pscrp, psmel,
               id_sb, gate_sb, band_sb, bw_sb, ones2,
               sa, sd, spst, cs_ps, d_cv, d_mel, d_chunks):
    nb = band_sb.shape[1]

    # --- gate BCE: Abs + Softplus on ACT (both live in the same activation
    # table set as mel's Abs -> a single table load for the whole program);
    # x*z and relu sums on DVE ---
    go = gate_sb[:, 0:GCOLS]
    gt = gate_sb[:, GCOLS:2 * GCOLS]
    g1 = sp.tile([128, GCOLS], F32, tag="g1")
    nc.scalar.activation(out=g1[:], in_=go, func=ACTF.Abs)
    g2 = sp.tile([128, GCOLS], F32, tag="g2")
    nc.scalar.activation(out=g2[:], in_=g1[:], func=ACTF.Exp, scale=-1.0)
    g3 = sp.tile([128, GCOLS], F32, tag="g3")
    nc.scalar.activation(out=g3[:], in_=g2[:], func=ACTF.Ln, bias=1.0,
                         accum_out=sa[:, 10:11])
    g5 = sp.tile([128, GCOLS], F32, tag="g5")
    nc.vector.scalar_tensor_tensor(
        out=g5[:], in0=go, scalar=0.0, in1=gt,
        op0=ALU.add, op1=ALU.mult, accum_out=spst[:, 0:1])
    g6 = sp.tile([128, GCOLS], F32, tag="g6")
    nc.vector.scalar_tensor_tensor(
        out=g6[:], in0=go, scalar=0.0, in1=go,
        op0=ALU.is_gt, op1=ALU.mult, accum_out=spst[:, 1:2])
    bscr = sp.tile([128, nb], BF16, tag="bscr")
    nc.vector.scalar_tensor_tensor(
        out=bscr[:], in0=band_sb, scalar=1.0, in1=bw_sb,
        op0=ALU.mult, op1=ALU.mult, accum_out=spst[:, 2:3])

    ones_v = ones2[:].rearrange("p (two s) -> p two s", two=2)[:, :, 0:1]
    id2 = id_sb.rearrange("p (two m) -> p two m", two=2)

    # --- mel L1 (PE diffs -> ACT/DVE abs+row-sum), canvas colsum chain
    #     interleaved so mel readers start as early as possible ---
    total_cv = d_chunks * 512
    cv_off = [0]
    cv_pair = [0]

    n_full_pairs = d_chunks // 2
    last_is_single = d_chunks % 2 == 1
    n_groups = n_full_pairs + (1 if last_is_single else 0)
    ones1 = ones2[:, 0:1]

    def emit_canvas_dma():
        if cv_off[0] >= total_cv:
            return
        w = min(2048, total_cv - cv_off[0])
        cvt = cvp.tile([128, 2048], F8, tag="cv")
        nc.sync.dma_start(out=cvt[:, 0:w], in_=d_cv[:, cv_off[0]:cv_off[0] + w])
        h = 0
        while h * 1024 < w:
            first = cv_pair[0] == 0
            last = cv_pair[0] == n_groups - 1
            if w - h * 1024 >= 1024:
                nc.tensor.matmul(
                    cs_ps[:], ones_v,
                    cvt[:, h * 1024:(h + 1) * 1024].rearrange(
                        "p (two j) -> p two j", two=2),
                    start=first, stop=last,
                    perf_mode=DR, skip_group_check=True)
            else:
                # odd trailing 512-col chunk: plain fp8 matmul
                nc.tensor.matmul(
                    cs_ps[:], ones1, cvt[:, h * 1024:h * 1024 + 512],
                    start=first, stop=last, skip_group_check=True)
            cv_pair[0] += 1
            h += 1
        cv_off[0] += w

    n_cv_dmas = (total_cv + 2047) // 2048
    # canvas DMA schedule: spread between mel half-tiles (mel first)
    cv_after = [1, 1, 1, 1, 1, 1, 1, 0, 0, 0]
    rem = n_cv_dmas - sum(cv_after)
    cv_after[6] += max(rem, 0)

    ncols = {'A': 0, 'D': 0}
    for kh in range(NMT * 2):
        mt = melp.tile([128, MELW], F8, tag="mel")
        nc.sync.dma_start(out=mt[:], in_=d_mel[:, kh * MELW:(kh + 1) * MELW])
        for p in range(2):
            # pair 0: planes (mo, mt) -> mo - mt; pair 1: (mt, mp) -> mt - mp
            pv = mt[:, p * MELS:p * MELS + 2 * MELS].rearrange(
                "p (two j) -> p two j", two=2)
            ps = psmel.tile([128, 1024], F32, tag="mps")
            nc.tensor.matmul(ps[:, 0:512], id2, pv[:, :, 0:512],
                             start=True, stop=True, perf_mode=DR,
                             skip_group_check=True)
            nc.tensor.matmul(ps[:, 512:MELH], id2, pv[:, :, 512:MELH],
                             start=True, stop=True, perf_mode=DR,
                             skip_group_check=True)
            eng = READER[kh * 2 + p]
            col = ncols[eng]
            ncols[eng] += 1
            if eng == 'A':
                scr = scrp.tile([128, MELH], BF16, tag="scr")
                nc.scalar.activation(out=scr[:], in_=ps[:, 0:MELH],
                                     func=ACTF.Abs,
                                     accum_out=sa[:, col:col + 1])
            else:
                nc.vector.tensor_reduce(
                    out=sd[:, col:col + 1], in_=ps[:, 0:MELH],
                    axis=mybir.AxisListType.X, op=ALU.add,
                    apply_absolute_value=True)
        if kh < len(cv_after):
            for _ in range(cv_after[kh]):
                emit_canvas_dma()
    while cv_off[0] < total_cv:
        emit_canvas_dma()


_PROGRAMS = {}


def _get_program(d_chunks=None, nb=None, n_reps=1):
    if d_chunks is None or nb is None:
        assert _LAYOUT is not None, "call kernel() first"
        d_chunks, _, nb = _LAYOUT
    key = (d_chunks, nb, n_reps)
    if key not in _PROGRAMS:
        _PROGRAMS[key] = _build_program(d_chunks, nb, n_reps)
    return _PROGRAMS[key]


def _build_program_reps(n_reps):
    assert _LAYOUT is not None, "call kernel() (or _prep_core) first"
    d, _, nb = _LAYOUT
    return _get_program(d, nb, n_reps)


def _core_box_count(in_len, out_len):
    return int(np.sum(out_len.astype(np.int64) * (TIN - in_len.astype(np.int64))))


def _core_band_cols(out_len):
    return -(-int(np.sum(out_len.astype(np.int64))) * BW // 128)


def _prep_core(al, melo, melp_, melt, go, gt, in_len, out_len):
    """Build one core's input map. al: [BPC, TOUT, TIN] etc. (numpy f32)."""
    global _LAYOUT
    in_len = np.asarray(in_len, dtype=np.int64)
    out_len = np.asarray(out_len, dtype=np.int64)
    if _LAYOUT is None:
        # standalone use: size from this core with margin
        _LAYOUT = _canvas_layout(int(_core_box_count(in_len, out_len) * 1.25),
                                 _core_band_cols(out_len) + 64)
    d, ranges, nb = _LAYOUT

    # mel: per (row-tile k, half h): [mo | mt | mp] planes of MELH cols
    # padded to MELS so the DoubleRow plane stride is a multiple of 16
    m3 = np.stack([melo.reshape(MROWS, TOUT),
                   melt.reshape(MROWS, TOUT),
                   melp_.reshape(MROWS, TOUT)], axis=1)     # [640, 3, 2000]
    m4 = np.zeros((NMT, 128, 2, 3, MELS), np.float32)
    m5 = m3.reshape(NMT, 128, 3, 2, MELH)                   # [k, p, t, h, j]
    m4[:, :, :, :, 0:MELH] = m5.transpose(0, 1, 3, 2, 4)
    mel8 = np.ascontiguousarray(
        m4.transpose(1, 0, 2, 3, 4).reshape(128, NMT * 2 * MELW)).astype(F8NP)

    # canvas groups
    box_vals = [al[s, :out_len[s], in_len[s]:] for s in range(BPC)]
    box = (np.concatenate([v.ravel() for v in box_vals])
           if box_vals else np.zeros(0, np.float32))
    dirv = np.concatenate([al[s, :IMID + 1, :IMID][_DIR_MASK]
                           for s in range(BPC)])
    tailv = np.concatenate([al[s, IMID + 1:TIN, :][_TAIL_MASK]
                            for s in range(BPC)])

    cv = np.zeros((d, 512, 128), np.float32)
    for vals, (a, b) in zip((box, dirv, tailv), ranges):
        cap = d * (b - a) * 128
        assert len(vals) <= cap, f"canvas overflow: {len(vals)} > {cap}"
        pad = np.zeros(cap, np.float32)
        pad[:len(vals)] = vals * ASCALE
        cv[:, a:b, :] = pad.reshape(d, b - a, 128)
    cv8 = np.ascontiguousarray(cv.transpose(2, 0, 1).reshape(128, d * 512)
                               ).astype(F8NP)

    # gaussian band: 4 columns around j* = i*in/out for valid rows
    bands = []
    bws = []
    for s in range(BPC):
        ol, il = int(out_len[s]), int(in_len[s])
        iv = np.arange(ol, dtype=np.float64)
        jstar = iv * il / ol
        s0 = np.clip(np.floor(jstar).astype(np.int64) - 1, 0, TIN - BW)
        jb = s0[:, None] + np.arange(BW)[None, :]            # [ol, BW]
        bands.append(al[s, iv.astype(np.int64)[:, None], jb].ravel())
        dlt = iv[:, None] - jb * (float(ol) / il)
        w = np.exp(ESCALE * dlt * dlt)
        w[jb >= il] = 0.0
        bws.append(w.ravel())
    bflat = np.concatenate(bands)
    wflat = np.concatenate(bws)
    bpad = np.zeros(128 * nb, np.float32)
    bpad[:len(bflat)] = bflat
    wpad = np.zeros(128 * nb, np.float32)
    wpad[:len(wflat)] = wflat

    # identity stationary: [p, 0*128+m]=+1[p==m], [p, 128+m]=-1[p==m]
    idw = np.zeros((128, 256), np.float32)
    idw[np.arange(128), np.arange(128)] = 1.0
    idw[np.arange(128), 128 + np.arange(128)] = -1.0

    gate = np.ascontiguousarray(
        np.concatenate([go.reshape(128, GCOLS), gt.reshape(128, GCOLS)],
                       axis=1), dtype=np.float32)
    u8 = np.uint8
    aux = np.concatenate([
        np.ascontiguousarray(idw.astype(F8NP)).view(u8),
        gate.view(u8),
        np.ascontiguousarray(bpad.reshape(128, nb).astype(BFNP)).view(u8),
        np.ascontiguousarray(wpad.reshape(128, nb).astype(BFNP)).view(u8),
    ], axis=1)

    return {"aux": np.ascontiguousarray(aux), "mel": mel8, "cv": cv8}


def kernel(mel_out, mel_out_postnet, gate_out, alignments,
           mel_target, gate_target, input_lengths, output_lengths,
           _results_hook=None):
    global _LAYOUT
    mel_out = np.asarray(mel_out, dtype=np.float32)
    mel_out_postnet = np.asarray(mel_out_postnet, dtype=np.float32)
    gate_out = np.asarray(gate_out, dtype=np.float32)
    alignments = np.asarray(alignments, dtype=np.float32)
    mel_target = np.asarray(mel_target, dtype=np.float32)
    gate_target = np.asarray(gate_target, dtype=np.float32)
    in_len = np.asarray(input_lengths).astype(np.int64)
    out_len = np.asarray(output_lengths).astype(np.int64)

    # global layout from all cores (one SPMD program)
    max_box = 0
    max_nb = 0
    for c in range(NCORES):
        sl = slice(BPC * c, BPC * (c + 1))
        max_box = max(max_box, _core_box_count(in_len[sl], out_len[sl]))
        max_nb = max(max_nb, _core_band_cols(out_len[sl]))
    lay = _canvas_layout(max_box, max_nb)
    if _LAYOUT is None or _LAYOUT[0] < lay[0] or _LAYOUT[2] < lay[2]:
        _LAYOUT = lay
    d, ranges, nb = _LAYOUT

    in_maps = []
    for c in range(NCORES):
        sl = slice(BPC * c, BPC * (c + 1))
        in_maps.append(_prep_core(
            alignments[sl], mel_out[sl], mel_out_postnet[sl], mel_target[sl],
            gate_out[sl], gate_target[sl], in_len[sl], out_len[sl]))

    nc = _get_program(d, nb)
    res = run_bass_kernel_spmd(nc, in_maps, core_ids=list(range(NCORES)))
    if _results_hook is not None:
        _results_hook(res)

    mel_sum = gsp = grelu = gxz = gauss = 0.0
    att = box = 0.0
    (ba, bb), (da, db), (ta, tb) = ranges
    for c in range(NCORES):
        out = res.results[c]
        sa = out["sa"].astype(np.float64)
        sd = out["sd"].astype(np.float64)
        spst = out["sp"].astype(np.float64)
        cs = out["cs"].astype(np.float64)[0]

        mel_sum += sa[:, 0:10].sum() + sd[:, 0:10].sum()
        gsp += sa[:, 10].sum()
        gxz += spst[:, 0].sum()
        grelu += spst[:, 1].sum()
        gauss += spst[:, 2].sum()

        box_tail = cs[ba:bb].sum() / ASCALE
        att_dir = cs[da:db].sum() / ASCALE
        att_tail = cs[ta:tb].sum() / ASCALE

        sl = slice(BPC * c, BPC * (c + 1))
        att += BPC * ATT_CONST + att_dir - att_tail
        box += float(out_len[sl].sum()) - box_tail

    n_mel = B * MEL * TOUT
    n_gate = B * TOUT
    mel_loss = mel_sum / n_mel
    gate_loss = (grelu - gxz + gsp) / n_gate
    att_loss = att / B
    ga_loss = (box - gauss) / B
    total = (MEL_W * mel_loss + GATE_W * gate_loss
             + ATT_W * att_loss + GA_W * ga_loss)
    f = np.float32
    return (f(total), f(mel_loss), f(gate_loss), f(att_loss), f(ga_loss))
